# revision 4
# baseline (speedup 1.0000x reference)
"""Trainium2 Bass kernel for nn_ExpertFFN (MoE routing, E=8 experts, top-2).

Math (matching the reference exactly):
  xt = x.reshape(N, H); logits = xt @ Wr + br; gates = softmax(logits)
  G, idx = top2(gates); P = one_hot(idx)           # (N,2), (N,2)
  tok[e,k,:]  = sum_n P[n,k,e] * xt[n,:]           # (E,2,H)
  mid[e] = gelu(tok[e] @ W1[e] + b1[e])            # exact erf-gelu
  Eo[e]  = mid[e] @ W2[e] + b2[e]                  # (E,2,H)
  y[n]   = sum_k G[n,k] * (sum_e Eo[e,k,:])        # combine over ALL experts

Sharding across 8 NeuronCores:
  - router + dispatch: token-parallel (1024 tokens/core), partial tok
    summed across cores with an in-kernel ReduceScatter -> core c holds
    tok rows [2c, 2c+2) = expert c's two aggregate tokens.
  - FFN: expert-parallel; core c streams W1[c], W2[c] (128 MiB) once.
  - combine: AllReduce of the tiny (2, H) expert outputs, then each core
    computes y for its own 1024 tokens.
The kernel is HBM-bound on the 1 GiB weight stream (memory regime).

Router (v3): logits for 4 token-tiles are accumulated into a single PSUM
bank [128, 4, 8], so softmax/top-2 runs once per group as innermost-axis
segment ops (~9 DVE/ACT instructions per 4 tiles instead of ~44), and the
dispatch uses bf16 copies of x so the 512-wide moving operand streams at
1 cycle/row. Constants load on the ACT HWDGE ring so the 2k+ tiny router
weight descriptors never stall the SP ring that feeds the x tiles.
"""

import numpy as np

import concourse.bass as bass
import concourse.bacc as bacc
import concourse.mybir as mybir
import concourse.tile as tile
from concourse.masks import make_identity
from concourse.bass_utils import run_bass_kernel_spmd

F32 = mybir.dt.float32
BF16 = mybir.dt.bfloat16
AX = mybir.AxisListType.X
ALU = mybir.AluOpType
ACT_GELU = mybir.ActivationFunctionType.Gelu
ACT_EXP = mybir.ActivationFunctionType.Exp

# problem dims
B, S, H, F, E, TOPK = 4, 2048, 2048, 8192, 8, 2
N = B * S
NCORES = 8
P = 128


def build_expert_ffn(nc_cores=NCORES, h=H, f=F, e=E, tn=None, w_bufs=6,
                     x_bufs=2, collectives=True):
    # bf16 datapath: fp32 HBM tensors are cast to bf16 during the (SWDGE)
    # DMA, so the PE streams the weight operand at 1 cycle/row instead of
    # fp32's 4, and SBUF weight slabs take half the space. All accumulation
    # stays fp32 in PSUM; the bf16 rounding (~0.2% relative) is far inside
    # the 2e-2 rel-err budget.
    tn = tn if tn is not None else N // nc_cores  # tokens per core
    nt = tn // P        # token tiles
    hc = h // P         # h chunks of 128
    ng = f // 512       # f groups
    ek = e * TOPK       # rows of tok
    hb = h // 512       # output column banks

    nc = bacc.Bacc("TRN2", target_bir_lowering=False, debug=False,
                   num_devices=nc_cores)

    xs = nc.dram_tensor("xs", [tn, h], F32, kind="ExternalInput")
    Wr = nc.dram_tensor("Wr", [h, e], F32, kind="ExternalInput")
    brb = nc.dram_tensor("brb", [1, e], F32, kind="ExternalInput")
    W1c = nc.dram_tensor("W1c", [h, f], F32, kind="ExternalInput")
    b1c = nc.dram_tensor("b1c", [1, f], F32, kind="ExternalInput")
    W2c = nc.dram_tensor("W2c", [f, h], F32, kind="ExternalInput")
    b2c = nc.dram_tensor("b2c", [1, h], F32, kind="ExternalInput")
    yo = nc.dram_tensor("y", [tn, h], F32, kind="ExternalOutput")

    groups = [list(range(nc_cores))]

    with tile.TileContext(nc) as tc:
        with (
            tc.tile_pool(name="const", bufs=1) as cpool,
            tc.tile_pool(name="sb", bufs=1) as sb,
            tc.tile_pool(name="xpool", bufs=x_bufs) as xpool,
            tc.tile_pool(name="xbpool", bufs=5) as xbpool,
            tc.tile_pool(name="xtpool", bufs=4) as xtpool,
            tc.tile_pool(name="small", bufs=2) as small,
            tc.tile_pool(name="thpool", bufs=1) as thpool,
            tc.tile_pool(name="wpool", bufs=w_bufs) as wpool,
            tc.tile_pool(name="dram", bufs=1, space="DRAM") as dram,
        ):
            # ---- constants (ACT HWDGE ring: keep the SP ring free for the
            # latency-critical x tiles) ----
            ident = cpool.tile([P, P], F32)
            make_identity(nc, ident[:])
            ones1 = cpool.tile([1, P], F32)
            nc.gpsimd.memset(ones1[:], 1.0)
            br4 = cpool.tile([1, 4, e], F32)
            for tt in range(4):
                nc.scalar.dma_start(br4[:, tt, :], brb[:])
            wr_sb = cpool.tile([P, hc, e], F32)
            for j in range(hc):
                nc.scalar.dma_start(wr_sb[:, j, :], Wr[j * P:(j + 1) * P, :])
            b2_sb = cpool.tile([2, h], F32)
            for r in range(2):
                nc.scalar.dma_start(b2_sb[r:r + 1, :], b2c[:])
            # b1 in col-tiled layout: rows 32q+r hold b1[q*(f/4) : (q+1)*(f/4)]
            fq = f // 4
            b1t = cpool.tile([P, fq], F32)
            b1v = b1c.ap().rearrange("o (q i) -> o q i", q=4)
            for q in range(4):
                for r in range(2):
                    nc.scalar.dma_start(b1t[32 * q + r:32 * q + r + 1, :],
                                        b1v[:, q, :])

            # persistent router outputs
            gv_all = sb.tile([P, nt, 2], F32)      # top-2 gate values
            tokp_sb = sb.tile([ek, h], F32)        # partial tok (this core)

            # prefetch the first W1 slabs NOW — no deps, so the weight
            # stream starts at t=0 and overlaps the whole router phase
            w1v = W1c.ap().rearrange("(j p) f -> p j f", p=P)
            w_pre = []
            for j in range(w_bufs):
                w1s = wpool.tile([P, f], BF16, tag="w", name=f"w1pre{j}")
                nc.gpsimd.dma_start(w1s[:], w1v[:, j, :])
                w_pre.append(w1s)

            # ================= phase R: router + dispatch =================
            with (
                tc.tile_pool(name="ps_r", bufs=1, space="PSUM") as ps_r,
                tc.tile_pool(name="ps_tok", bufs=1, space="PSUM") as ps_tok,
            ):
                # dispatch psum, col-tiled: h-bank b lives at partitions
                # [32b, 32b+ek) of one (128, 512) bank
                tokp = ps_tok.tile([P, 512], F32)
                ngrp = nt // 4
                for g in range(ngrp):
                    # logits for 4 tiles accumulate into one PSUM bank,
                    # pre-seeded with the router bias via a K=1 outer product
                    lgb = ps_r.tile([P, 4, e], F32, tag="lg", bufs=2)
                    nc.tensor.matmul(
                        lgb[:].rearrange("p t e -> p (t e)"),
                        ones1[:], br4[:], start=True, stop=False)
                    xbs = []
                    for tt in range(4):
                        t = 4 * g + tt
                        x_t = xpool.tile([P, h], F32, tag="x")
                        nc.sync.dma_start(x_t[:], xs[t * P:(t + 1) * P, :])
                        xb = xbpool.tile([P, h], BF16, tag="xb")
                        nc.vector.tensor_copy(xb[:], x_t[:])
                        xbs.append(xb)
                        for jb in range(4):
                            xt_ps = ps_r.tile([P, 4 * P], F32, tag="xtps",
                                              bufs=2)
                            for jj in range(4):
                                j = 4 * jb + jj
                                nc.tensor.transpose(
                                    xt_ps[:, jj * P:(jj + 1) * P],
                                    x_t[:, j * P:(j + 1) * P], ident[:])
                            xt_sb = xtpool.tile([P, 4 * P], F32, tag="xt")
                            nc.vector.tensor_copy(xt_sb[:], xt_ps[:])
                            for jj in range(4):
                                j = 4 * jb + jj
                                nc.tensor.matmul(
                                    lgb[:, tt, :],
                                    xt_sb[:, jj * P:(jj + 1) * P],
                                    wr_sb[:, j, :], start=False,
                                    stop=(j == hc - 1))
                    # ---- segment softmax + top-2 over the 4-tile group.
                    # No max-subtraction: |logits| < ~6 for this model
                    # (x ~ N(0,1), Wr ~ 0.02*N(0,1), H=2048), exp is safe.
                    gslc = gv_all[:, 4 * g:4 * g + 4, :]
                    ex = small.tile([P, 4, e], F32, tag="ex4")
                    nc.scalar.activation(
                        ex[:].rearrange("p t e -> p (t e)"),
                        lgb[:].rearrange("p t e -> p (t e)"), ACT_EXP)
                    ssum = small.tile([P, 4], F32, tag="ssum4")
                    nc.vector.reduce_sum(ssum[:], ex[:], axis=AX)
                    rinv = small.tile([P, 4], F32, tag="rinv4")
                    nc.vector.reciprocal(rinv[:], ssum[:])
                    msel = small.tile([P, 4, e, 2], BF16, tag="msel4")
                    m1 = gslc[:, :, 0:1]
                    nc.vector.reduce_max(m1, ex[:], axis=AX)
                    nc.vector.tensor_tensor(
                        msel[:, :, :, 0], ex[:],
                        m1.to_broadcast([P, 4, e]), ALU.is_equal)
                    m1m = small.tile([P, 4, e], F32, tag="m1m4")
                    nc.vector.tensor_scalar(m1m[:], msel[:, :, :, 0], -2.0,
                                            1.0, op0=ALU.mult, op1=ALU.add)
                    ex2 = small.tile([P, 4, e], F32, tag="ex24")
                    nc.vector.tensor_tensor(ex2[:], ex[:], m1m[:], ALU.mult)
                    m2 = gslc[:, :, 1:2]
                    nc.vector.reduce_max(m2, ex2[:], axis=AX)
                    nc.vector.tensor_tensor(
                        msel[:, :, :, 1], ex2[:],
                        m2.to_broadcast([P, 4, e]), ALU.is_equal)
                    for kk in range(2):
                        nc.vector.tensor_tensor(
                            gslc[:, :, kk], gslc[:, :, kk], rinv[:], ALU.mult)
                    # ---- dispatch (bf16): tokp += msel.T @ xb ----
                    for tt in range(4):
                        t = 4 * g + tt
                        msel2 = msel[:, tt, :, :].rearrange("p e k -> p (e k)")
                        for b in range(hb):
                            nc.tensor.matmul(
                                tokp[32 * b:32 * b + ek, :], msel2,
                                xbs[tt][:, b * 512:(b + 1) * 512],
                                start=(t == 0), stop=(t == nt - 1),
                                tile_position=(0, 32 * b))
                for b in range(hb):
                    nc.vector.tensor_copy(tokp_sb[:, b * 512:(b + 1) * 512],
                                          tokp[32 * b:32 * b + ek, :])

            # ======== ReduceScatter: sum tok over cores, keep own expert ====
            cc1_in = dram.tile([ek, h], F32)
            cc1_out = dram.tile([TOPK, h], F32)
            nc.sync.dma_start(cc1_in[:], tokp_sb[:])
            if collectives:
                nc.gpsimd.collective_compute(
                    "ReduceScatter", ALU.add, replica_groups=groups,
                    ins=[cc1_in.opt()], outs=[cc1_out.opt()])
            else:
                nc.sync.dma_start(cc1_out[:], cc1_in[0:TOPK, :])
            tokc = thpool.tile([TOPK, h], F32, tag="th")
            nc.sync.dma_start(tokc[:], cc1_out[:])

            # tokT: (h, 2) laid out as hc chunks of (128, 2)
            tokT = sb.tile([P, hc, 2], BF16)
            with tc.tile_pool(name="ps_f", bufs=1, space="PSUM") as ps_f:
                for j in range(hc):
                    tt_ps = ps_f.tile([P, 2], F32, tag="tp", bufs=2)
                    nc.tensor.transpose(tt_ps[:], tokc[:, j * P:(j + 1) * P],
                                        ident[:2, :2])
                    nc.vector.tensor_copy(tokT[:, j, :], tt_ps[:])

                # G^T for the final combine (only needs gv_all; do it early
                # so it is off the post-AllReduce critical path)
                gt_all = sb.tile([TOPK, nt, P], BF16)
                for t in range(nt):
                    gt_ps = ps_f.tile([TOPK, P], F32, tag="tp", bufs=2)
                    nc.tensor.transpose(gt_ps[:], gv_all[:, t, :], ident[:])
                    nc.vector.tensor_copy(gt_all[:, t, :], gt_ps[:])

                # ================= phase F: expert FFN =================
                # per-half psum accumulators in SEPARATE banks so the
                # half-0 DVE evacuation never touches the bank half-1's
                # matmuls are still writing (bank-aware serialization would
                # otherwise kill the AR/stream overlap)
                psum_yh = [ps_f.tile([P, 512], F32, name=f"psum_y{i}")
                           for i in range(2)]
                w2v = W2c.ap().rearrange("(g q p) f -> p g q f", q=4, p=P)
                fc = f // P          # 64 f-chunks of 128
                fcq = fc // 4        # 16 f-chunks per col group
                midg = sb.tile([P, fq], F32)       # gelu(mid), col-tiled
                midT = sb.tile([P, fc, 2], BF16)   # mid^T chunks (lhsT for W2)

                # ---- W1: h-major contiguous slabs; mid col-tiled:
                # col group q holds f in [q*fq, (q+1)*fq) ----
                # mid_ps gets its own pool so its 4 banks free before the
                # combine's yt pool opens (PSUM budget: 8 banks total)
                with tc.tile_pool(name="ps_mid", bufs=1,
                                  space="PSUM") as ps_mid:
                    mid_ps = ps_mid.tile([P, fq], F32, tag="mid")
                    for j in range(hc):
                        if j < len(w_pre):
                            w1s = w_pre[j]
                        else:
                            w1s = wpool.tile([P, f], BF16, tag="w")
                            nc.gpsimd.dma_start(w1s[:], w1v[:, j, :])
                        for q in range(4):
                            for nb in range(fq // 512):
                                sl = slice(nb * 512, (nb + 1) * 512)
                                nc.tensor.matmul(
                                    mid_ps[32 * q:32 * q + 2, sl],
                                    tokT[:, j, :],
                                    w1s[:, q * fq + nb * 512:
                                        q * fq + (nb + 1) * 512],
                                    start=(j == 0), stop=(j == hc - 1),
                                    tile_position=(0, 32 * q))
                    # bias + exact gelu on the whole mid at once
                    nc.vector.tensor_add(midg[:], mid_ps[:], b1t[:])
                    nc.scalar.activation(midg[:], midg[:], ACT_GELU)
                # transpose mid chunks: global f-chunk k = q*fcq + m
                for q in range(4):
                    for m in range(fcq):
                        mt_ps = ps_f.tile([P, 2], F32, tag="tp", bufs=2)
                        nc.tensor.transpose(
                            mt_ps[:], midg[32 * q:32 * q + 2,
                                           m * P:(m + 1) * P],
                            ident[32 * q:32 * q + 2, 32 * q:32 * q + 2],
                            tile_position=(32 * q, 0))
                        nc.vector.tensor_copy(midT[:, q * fcq + m, :],
                                              mt_ps[:])

                # ---- W2 in h-halves: Eo[:, half] completes while the
                # other half is still streaming, so its AllReduce and half
                # the combine hide under the remaining W2 DMA ----
                hh2 = h // 2
                w2vh = W2c.ap().rearrange("(g q p) (hh m) -> p g q hh m",
                                          q=4, p=P, hh=2)
                with tc.tile_pool(name="ps_c", bufs=1, space="PSUM") as ps_c:
                    for hhi in range(2):
                        for g in range(ng):
                            w2s = wpool.tile([P, 4, hh2], BF16, tag="w")
                            nc.gpsimd.dma_start(w2s[:], w2vh[:, g, :, hhi, :])
                            for ft in range(4):
                                for qq in range(2):
                                    nc.tensor.matmul(
                                        psum_yh[hhi][32 * qq:32 * qq + 2, :],
                                        midT[:, g * 4 + ft, :],
                                        w2s[:, ft, qq * 512:(qq + 1) * 512],
                                        start=(g == 0 and ft == 0),
                                        stop=(g == ng - 1 and ft == 3),
                                        tile_position=(0, 32 * qq))

                        eo_h = thpool.tile([TOPK, hh2], F32, tag="th")
                        for qq in range(2):
                            nc.vector.tensor_copy(
                                eo_h[:, qq * 512:(qq + 1) * 512],
                                psum_yh[hhi][32 * qq:32 * qq + 2, :])
                        nc.vector.tensor_add(
                            eo_h[:], eo_h[:],
                            b2_sb[:, hhi * hh2:(hhi + 1) * hh2])

                        # ---- AllReduce this half: A_h = sum_e Eo_h[e] ----
                        cc2_in = dram.tile([TOPK, hh2], F32)
                        cc2_out = dram.tile(
                            [TOPK, hh2], F32,
                            addr_space="Shared" if collectives else "Local")
                        nc.sync.dma_start(cc2_in[:], eo_h[:])
                        if collectives:
                            nc.gpsimd.collective_compute(
                                "AllReduce", ALU.add, replica_groups=groups,
                                ins=[cc2_in.opt()], outs=[cc2_out.opt()])
                        else:
                            nc.sync.dma_start(cc2_out[:], cc2_in[:])
                        a_h = sb.tile([TOPK, hh2], F32, name=f"a_h{hhi}")
                        nc.sync.dma_start(a_h[:], cc2_out[:])
                        a_bf = sb.tile([TOPK, hh2], BF16, name=f"a_bf{hhi}")
                        nc.vector.tensor_copy(a_bf[:], a_h[:])

                        # ---- combine this half: y[:, half] = G @ A_h ----
                        for t in range(nt):
                            # reuse x slots (x tiles are dead after routing)
                            y_sb = xpool.tile([P, hh2], F32, tag="x")
                            for bb in range(2):
                                yt_ps = ps_c.tile([P, 512], F32, tag="yt",
                                                  bufs=2)
                                nc.tensor.matmul(
                                    yt_ps[:], gt_all[:, t, :],
                                    a_bf[:, bb * 512:(bb + 1) * 512],
                                    start=True, stop=True)
                                nc.vector.tensor_copy(
                                    y_sb[:, bb * 512:(bb + 1) * 512],
                                    yt_ps[:])
                            nc.sync.dma_start(
                                yo[t * P:(t + 1) * P,
                                   hhi * hh2:(hhi + 1) * hh2], y_sb[:])

    nc.compile()
    return nc


_NC_CACHE = {}


def _get_nc():
    if "nc" not in _NC_CACHE:
        _NC_CACHE["nc"] = build_expert_ffn()
    return _NC_CACHE["nc"]


def kernel(x, Wr, br, W1, b1, W2, b2):
    x = np.ascontiguousarray(np.asarray(x, dtype=np.float32))
    Wr = np.ascontiguousarray(np.asarray(Wr, dtype=np.float32))
    br = np.ascontiguousarray(np.asarray(br, dtype=np.float32))
    W1 = np.ascontiguousarray(np.asarray(W1, dtype=np.float32))
    b1 = np.ascontiguousarray(np.asarray(b1, dtype=np.float32))
    W2 = np.ascontiguousarray(np.asarray(W2, dtype=np.float32))
    b2 = np.ascontiguousarray(np.asarray(b2, dtype=np.float32))

    nc = _get_nc()
    tn = N // NCORES
    x2 = x.reshape(N, H)
    in_maps = []
    for c in range(NCORES):
        in_maps.append({
            "xs": x2[c * tn:(c + 1) * tn],
            "Wr": Wr,
            "brb": br.reshape(1, E),
            "W1c": W1[c],
            "b1c": b1[c].reshape(1, F),
            "W2c": W2[c],
            "b2c": b2[c].reshape(1, H),
        })
    # The very first execution after a fresh mesh/NEFF load has been seen
    # to return garbage on this runtime; run once to warm up, then run the
    # call that produces the returned result. Only the first kernel() call
    # in a process pays this.
    if "warm" not in _NC_CACHE:
        run_bass_kernel_spmd(nc, in_maps, list(range(NCORES)), trace=False)
        _NC_CACHE["warm"] = True
    res = run_bass_kernel_spmd(nc, in_maps, list(range(NCORES)), trace=False)
    y = np.concatenate([res.results[c]["y"] for c in range(NCORES)], axis=0)
    return y.reshape(B, S, H)
